# revision 30
# baseline (speedup 1.0000x reference)
"""Trainium2 Bass kernel for the DQN-style network.

Shapes (hardcoded): N=256, D=128, H=64, L=16, V=128, A=40,
F3_IN = N*(D+H+N) = 114688, f3 hidden = 512.

Strategy (8 cores, SPMD, no collectives):
  - Row-shard f3_W1: core c owns rows [64c, 64c+64).  Each core computes
    h1_c = relu(W1_c @ x + b1_c) fully locally, then its partial output
    W2[:, rows_c] @ h1_c (b2 added on core 0 only via its input data).
    The host sums the 8 partial [40] vectors (unshard of a sum-sharded
    output).
  - The tiny encoder (GRU over 16 tokens) and the rank-1 factorized
    global softmax p = softmax(u) (x) softmax(v) are computed redundantly
    on every core and overlap with the weight streaming.
  - x is built on-chip as SBUF tiles whose columns are the 128-long
    K-chunks of x; the host pre-permutes W1_c^T rows into the same chunk
    order so the accumulating matmuls (lhsT = x column(s), rhs = W chunk
    [128,64]) stream the weight at full DMA rate.
  - Mixed precision keeps the streamed bytes at 11.6 MB/core (vs 29.4
    f32) with ~6e-5 output error: the error budget is dominated by the
    obs block (x values O(1)), so obs weights stream as bf16-hi plus a
    2^13-scaled fp8 residual (with x split hi/lo bf16 + fp8), while the
    g/p blocks (tiny contributions) stream as scaled fp8.  Partial sums
    accumulate in three f32 PSUM accumulators (scales 1, 2^13, 2^24)
    that are descaled and combined in the tail.
"""

from contextlib import ExitStack

import numpy as np

import concourse.bass as bass
import concourse.mybir as mybir
from concourse import bacc
from concourse.bass_utils import run_bass_kernel_spmd
from concourse.masks import make_identity
from concourse.tile import TileContext

N, D, H, L, V, A = 256, 128, 64, 16, 128, 40
HID = 512                      # f3 hidden
RPC = HID // 8                 # rows of W1 per core = 64
NCHUNK = 896                   # 114688 / 128
NGROUP = 28                    # chunk groups of 32 (1 MB DMA each)
F32 = mybir.dt.float32
BF16 = mybir.dt.bfloat16
FP8 = mybir.dt.float8e4
FX = np.float32
LO_SCALE = 2.0 ** 13           # obs-residual fp8 pre-scale
G_WSCALE, G_XSCALE = 256.0, 32.0     # g-part fp8 pre-scales
P_WSCALE, P_XSCALE = 256.0, 65536.0  # p-part fp8 pre-scales

# blob free-dim column offsets: name -> (col0, width, rows)
_BLOB_FIELDS = [
    ("obsT", N, 128), ("embA", L, H + 1), ("wihA", 3 * H, H + 1),
    ("whhR", H, H + 1), ("whhZ", H, H + 1), ("whhN", H, H + 1),
    ("f1w", 2 * D, H), ("eyep", 128, H), ("b1row", RPC, 1),
    ("w2t", A, RPC), ("b2col", 1, A),
]
_BLOB = {}
_c = 0
for _nm, _w, _r in _BLOB_FIELDS:
    _BLOB[_nm] = (_c, _w, _r)
    _c += _w
BLOBW = _c

_CACHE = {}


def _build_module():
    nc = bacc.Bacc()

    wth = nc.declare_dram_parameter("wth", [2, 128, 8192], BF16, isOutput=False)
    wtl = nc.declare_dram_parameter("wtl", [1, 128, 16384], FP8, isOutput=False)
    wtg = nc.declare_dram_parameter("wtg", [1, 128, 8192], FP8, isOutput=False)
    wtp = nc.declare_dram_parameter("wtp", [2, 128, 16384], FP8, isOutput=False)
    obsHL = nc.declare_dram_parameter("obsHL", [128, 2 * N], BF16, isOutput=False)
    obs8 = nc.declare_dram_parameter("obs8", [128, N], FP8, isOutput=False)
    # all small f32 inputs packed into one [128, BLOBW] tensor (col ranges
    # in _BLOB): obsT | embA | wihA | whhR | whhZ | whhN | f1w | eyep |
    # b1row | w2t | b2col
    blob = nc.declare_dram_parameter("blob", [128, BLOBW], F32, isOutput=False)
    out = nc.declare_dram_parameter("out", [A, 1], F32, isOutput=True)

    AF = mybir.ActivationFunctionType
    OP = mybir.AluOpType

    with TileContext(nc) as tc, ExitStack() as st:
        cst = st.enter_context(tc.tile_pool(name="cst", bufs=1))
        tmp = st.enter_context(tc.tile_pool(name="tmp", bufs=2))
        wsp = st.enter_context(tc.tile_pool(name="wsp", bufs=6))
        psa = st.enter_context(tc.tile_pool(name="psa", bufs=1, space="PSUM"))
        psp = st.enter_context(tc.tile_pool(name="psp", bufs=4, space="PSUM"))

        # ---- constants / small input loads ----
        ident = cst.tile([128, 128], F32)
        make_identity(nc, ident[:, :])
        ones = cst.tile([128, 128], F32)
        nc.gpsimd.memset(ones[:, :], 1.0)

        blob_sb = cst.tile([128, BLOBW], F32)
        nc.sync.dma_start(out=blob_sb[:, :], in_=blob[:, :])
        xhl = cst.tile([128, 2 * N], BF16)
        nc.sync.dma_start(out=xhl[:, :], in_=obsHL[:, :])
        x8 = cst.tile([128, N], FP8)
        nc.sync.dma_start(out=x8[:, :], in_=obs8[:, :])

        def bl(nm):
            c0, w, r = _BLOB[nm]
            return blob_sb[0:r, c0:c0 + w]

        xobs = bl("obsT")
        emb_sb = bl("embA")
        wih_sb = bl("wihA")
        whh_sb = {"r": bl("whhR"), "z": bl("whhZ"), "n": bl("whhN")}
        f1w_sb = bl("f1w")
        eyep_sb = bl("eyep")
        b1_sb = bl("b1row")
        w2t_sb = bl("w2t")
        b2_sb = bl("b2col")

        # ---- encoder: GI = [emb;1]^T @ [Wih^T;bih]  -> [L, 3H] ----
        ps_gi = psp.tile([L, 3 * H], F32, tag="ps")
        nc.tensor.matmul(ps_gi[:, :], emb_sb[:, :], wih_sb[:, :], start=True, stop=True)
        gi_sb = cst.tile([L, 3 * H], F32)
        nc.vector.tensor_copy(gi_sb[:, :], ps_gi[:, :])
        git = {}
        for j, nm in enumerate(("r", "z", "n")):
            ps_t = psp.tile([H, L], F32, tag="ps")
            nc.tensor.transpose(
                ps_t[:, :], gi_sb[:, j * H:(j + 1) * H], ident[0:L, 0:L]
            )
            g_sb = cst.tile([H, L], F32, tag=f"git_{nm}")
            nc.vector.tensor_copy(g_sb[:, :], ps_t[:, :])
            git[nm] = g_sb

        # ---- GRU: h kept as column in h_aug[0:64], h_aug[64] = 1 ----
        h_aug = cst.tile([H + 1, 1], F32)
        nc.gpsimd.memset(h_aug[:, :], 0.0)
        nc.gpsimd.memset(h_aug[H:H + 1, :], 1.0)
        for t in range(L):
            ps_r = psp.tile([H, 1], F32, tag="ps")
            nc.tensor.matmul(ps_r[:, :], whh_sb["r"][:, :], h_aug[:, :],
                             start=True, stop=True)
            r_s = tmp.tile([H, 1], F32, tag="r_s")
            nc.scalar.activation(r_s[:, :], ps_r[:, :], AF.Sigmoid,
                                 bias=git["r"][:, t:t + 1])
            ps_z = psp.tile([H, 1], F32, tag="ps")
            nc.tensor.matmul(ps_z[:, :], whh_sb["z"][:, :], h_aug[:, :],
                             start=True, stop=True)
            z_s = tmp.tile([H, 1], F32, tag="z_s")
            nc.scalar.activation(z_s[:, :], ps_z[:, :], AF.Sigmoid,
                                 bias=git["z"][:, t:t + 1])
            ps_n = psp.tile([H, 1], F32, tag="ps")
            nc.tensor.matmul(ps_n[:, :], whh_sb["n"][:, :], h_aug[:, :],
                             start=True, stop=True)
            # n = tanh(r * gh_n' + gi_n) in one ACT op (scale is [P,1] AP)
            n_s = tmp.tile([H, 1], F32, tag="n_s")
            nc.scalar.activation(n_s[:, :], ps_n[:, :], AF.Tanh,
                                 bias=git["n"][:, t:t + 1], scale=r_s[:, 0:1])
            t3 = tmp.tile([H, 1], F32, tag="t3")
            nc.vector.tensor_sub(t3[:, :], h_aug[0:H, :], n_s[:, :])
            # h' = (h - n) * z + n
            nc.vector.scalar_tensor_tensor(
                h_aug[0:H, :], t3[:, :], z_s[:, :], n_s[:, :],
                op0=OP.mult, op1=OP.add,
            )

        # ---- u = obs @ (Wa^T g), v = obs @ (Wb^T g) ----
        ps_wa = psp.tile([D, 1], F32, tag="ps")
        nc.tensor.matmul(ps_wa[:, :], f1w_sb[:, 0:D], h_aug[0:H, :],
                         start=True, stop=True)
        wa_s = tmp.tile([D, 1], F32, tag="wa_s")
        nc.vector.tensor_copy(wa_s[:, :], ps_wa[:, :])
        ps_wb = psp.tile([D, 1], F32, tag="ps")
        nc.tensor.matmul(ps_wb[:, :], f1w_sb[:, D:2 * D], h_aug[0:H, :],
                         start=True, stop=True)
        wb_s = tmp.tile([D, 1], F32, tag="wb_s")
        nc.vector.tensor_copy(wb_s[:, :], ps_wb[:, :])

        ps_u = psp.tile([128, 2], F32, tag="ps")
        nc.tensor.matmul(ps_u[:, 0:1], xobs[:, 0:128], wa_s[:, :],
                         start=True, stop=True)
        nc.tensor.matmul(ps_u[:, 1:2], xobs[:, 128:256], wa_s[:, :],
                         start=True, stop=True)
        eu = tmp.tile([128, 2], F32, tag="eu")
        nc.scalar.activation(eu[:, :], ps_u[:, :], AF.Exp)
        ps_v = psp.tile([128, 2], F32, tag="ps")
        nc.tensor.matmul(ps_v[:, 0:1], xobs[:, 0:128], wb_s[:, :],
                         start=True, stop=True)
        nc.tensor.matmul(ps_v[:, 1:2], xobs[:, 128:256], wb_s[:, :],
                         start=True, stop=True)
        ev = tmp.tile([128, 2], F32, tag="ev")
        nc.scalar.activation(ev[:, :], ps_v[:, :], AF.Exp)

        # global softmax normalizer: 1 / (sum(eu) * sum(ev))
        ps_su = psp.tile([1, 2], F32, tag="ps")
        nc.tensor.matmul(ps_su[:, :], ones[:, 0:1], eu[:, :], start=True, stop=True)
        ps_sv = psp.tile([1, 2], F32, tag="ps")
        nc.tensor.matmul(ps_sv[:, :], ones[:, 0:1], ev[:, :], start=True, stop=True)
        su_sb = tmp.tile([1, 2], F32, tag="su_sb")
        nc.vector.tensor_copy(su_sb[:, :], ps_su[:, :])
        sv_sb = tmp.tile([1, 2], F32, tag="sv_sb")
        nc.vector.tensor_copy(sv_sb[:, :], ps_sv[:, :])
        su_t = tmp.tile([1, 1], F32, tag="su_t")
        nc.vector.tensor_add(su_t[:, :], su_sb[:, 0:1], su_sb[:, 1:2])
        sv_t = tmp.tile([1, 1], F32, tag="sv_t")
        nc.vector.tensor_add(sv_t[:, :], sv_sb[:, 0:1], sv_sb[:, 1:2])
        sp_t = tmp.tile([1, 1], F32, tag="sp_t")
        nc.vector.tensor_mul(sp_t[:, :], su_t[:, :], sv_t[:, :])
        sc_t = tmp.tile([1, 1], F32, tag="sc_t")
        nc.vector.reciprocal(sc_t[:, :], sp_t[:, :])

        # rows: eu_row [1,256] scaled by sc; ev halves as [1,128] rows
        eu_row = tmp.tile([1, N], F32, tag="eu_row")
        for j in range(2):
            ps_t = psp.tile([1, 128], F32, tag="ps")
            nc.tensor.transpose(ps_t[:, :], eu[:, j:j + 1], ident[:, :])
            nc.vector.tensor_copy(eu_row[:, j * 128:(j + 1) * 128], ps_t[:, :])
        # fold the fp8 x-side pre-scale into the softmax normalizer
        eu_n = tmp.tile([1, N], F32, tag="eu_n")
        nc.vector.tensor_scalar(eu_n[:, :], eu_row[:, :], sc_t[:, 0:1],
                                P_XSCALE, op0=OP.mult, op1=OP.mult)
        ev_r = []
        for j in range(2):
            ps_t = psp.tile([1, 128], F32, tag="ps")
            nc.tensor.transpose(ps_t[:, :], ev[:, j:j + 1], ident[:, :])
            e_sb = tmp.tile([1, 128], F32, tag=f"ev_r{j}")
            nc.vector.tensor_copy(e_sb[:, :], ps_t[:, :])
            ev_r.append(e_sb)

        # p halves: outer products ev_half (x) (eu*sc*P_XSCALE), cast fp8
        xp8 = []
        for j in range(2):
            ps_p = psp.tile([128, N], F32, tag="ps")
            nc.tensor.matmul(ps_p[:, :], ev_r[j][:, :], eu_n[:, :],
                             start=True, stop=True)
            x_sb = cst.tile([128, N], FP8, tag=f"xp8_{j}")
            nc.vector.tensor_copy(x_sb[:, :], ps_p[:, :])
            xp8.append(x_sb)

        # g-pair tile: [g; g] * G_XSCALE broadcast to 128 columns, fp8
        ps_gp = psp.tile([128, 1], F32, tag="ps")
        nc.tensor.matmul(ps_gp[:, :], eyep_sb[:, :], h_aug[0:H, :],
                         start=True, stop=True)
        gcol = tmp.tile([128, 1], F32, tag="gcol")
        nc.vector.tensor_copy(gcol[:, :], ps_gp[:, :])
        xg8 = cst.tile([128, 128], FP8)
        nc.vector.tensor_scalar(xg8[:, :], ones[:, :], gcol[:, 0:1],
                                G_XSCALE, op0=OP.mult, op1=OP.mult)

        # ---- the big streamed matvec: 896 chunks of 128 ----
        # obs: W_hi bf16 (x hi+lo bf16 cols) -> ps_acc (scale 1)
        #      W_lo fp8*2^13 (x fp8)        -> ps8a (scale 2^13)
        # g:   W fp8*256 (x fp8*32)         -> ps8a (scale 2^13)
        # p:   W fp8*256 (x fp8*65536)      -> ps8b (scale 2^24)
        ps_acc = psa.tile([2, RPC], F32)
        ps8a = psa.tile([1, RPC], F32, tag="ps8a")
        ps8b = psa.tile([1, RPC], F32, tag="ps8b")
        for g in range(2):
            wth_t = wsp.tile([128, 8192], BF16, tag="ws")
            nc.sync.dma_start(out=wth_t[:, :], in_=wth[g, :, :])
            for j in range(128):
                c = 128 * g + j
                nc.tensor.matmul(
                    ps_acc[:, :], xhl[:, 2 * c:2 * c + 2],
                    wth_t[:, j * 64:(j + 1) * 64],
                    start=(c == 0), stop=(c == 255),
                )
        wtl_t = wsp.tile([128, 16384], FP8, tag="ws")
        nc.sync.dma_start(out=wtl_t[:, :], in_=wtl[0, :, :])
        for c in range(256):
            nc.tensor.matmul(
                ps8a[:, :], x8[:, c:c + 1], wtl_t[:, c * 64:(c + 1) * 64],
                start=(c == 0), stop=False,
            )
        for g in range(2):
            wtp_t = wsp.tile([128, 16384], FP8, tag="ws")
            nc.sync.dma_start(out=wtp_t[:, :], in_=wtp[g, :, :])
            for j in range(256):
                c = 256 * g + j
                xcol = xp8[0] if c < 256 else xp8[1]
                cc = c if c < 256 else c - 256
                nc.tensor.matmul(
                    ps8b[:, :], xcol[:, cc:cc + 1], wtp_t[:, j * 64:(j + 1) * 64],
                    start=(c == 0), stop=(c == 511),
                )
        wtg_t = wsp.tile([128, 8192], FP8, tag="ws")
        nc.sync.dma_start(out=wtg_t[:, :], in_=wtg[0, :, :])
        for j in range(128):
            nc.tensor.matmul(
                ps8a[:, :], xg8[:, j:j + 1], wtg_t[:, j * 64:(j + 1) * 64],
                start=False, stop=(j == 127),
            )

        # ---- tail: h1_c = relu(acc + b1_c); out = W2_c @ h1_c + b2 ----
        accs = tmp.tile([2, RPC], F32, tag="accs")
        nc.vector.tensor_copy(accs[:, :], ps_acc[:, :])
        ps_s = psp.tile([1, RPC], F32, tag="ps")
        nc.tensor.matmul(ps_s[:, :], ones[0:2, 0:1], accs[:, :],
                         start=True, stop=True)
        s1 = tmp.tile([1, RPC], F32, tag="s1")
        nc.vector.tensor_add(s1[:, :], ps_s[:, :], b1_sb[:, :])
        s2 = tmp.tile([1, RPC], F32, tag="s2")
        nc.vector.scalar_tensor_tensor(s2[:, :], ps8a[:, :], 1.0 / LO_SCALE,
                                       s1[:, :], op0=OP.mult, op1=OP.add)
        s3 = tmp.tile([1, RPC], F32, tag="s3")
        nc.vector.scalar_tensor_tensor(
            s3[:, :], ps8b[:, :], 1.0 / (P_WSCALE * P_XSCALE),
            s2[:, :], op0=OP.mult, op1=OP.add)
        h1r = tmp.tile([1, RPC], F32, tag="h1r")
        nc.scalar.activation(h1r[:, :], s3[:, :], AF.Relu)
        ps_h1 = psp.tile([RPC, 1], F32, tag="ps")
        nc.tensor.transpose(ps_h1[:, :], h1r[:, :], ident[0:1, 0:1])
        h1c = tmp.tile([RPC, 1], F32, tag="h1c")
        nc.vector.tensor_copy(h1c[:, :], ps_h1[:, :])
        ps_o = psp.tile([A, 1], F32, tag="ps")
        nc.tensor.matmul(ps_o[:, :], w2t_sb[:, :], h1c[:, :], start=True, stop=True)
        o_sb = tmp.tile([A, 1], F32, tag="o_sb")
        nc.vector.tensor_add(o_sb[:, :], ps_o[:, :], b2_sb[:, :])
        nc.sync.dma_start(out=out[:, :], in_=o_sb[:, :])

    nc.compile()
    return nc


def _prep_in_maps(obs, token_ids, emb_table, gru_Wih, gru_Whh, gru_bih,
                  gru_bhh, f1_W, f1_b, f3_W1, f3_b1, f3_W2, f3_b2):
    obs = np.asarray(obs, FX)
    ids = np.asarray(token_ids).astype(np.int64)
    emb = np.asarray(emb_table, FX)[ids]                     # [L, H]
    Wih = np.asarray(gru_Wih, FX)
    Whh = np.asarray(gru_Whh, FX)
    bih = np.asarray(gru_bih, FX)
    bhh = np.asarray(gru_bhh, FX)
    W1 = np.asarray(f3_W1, FX)
    W2 = np.asarray(f3_W2, FX)
    b1 = np.asarray(f3_b1, FX)
    b2 = np.asarray(f3_b2, FX)

    import ml_dtypes
    BF = ml_dtypes.bfloat16
    F8 = ml_dtypes.float8_e4m3

    obsT_f = np.ascontiguousarray(obs.T)
    obsH = obsT_f.astype(BF)
    obsL = (obsT_f - obsH.astype(FX)).astype(BF)
    fields = {
        "obsT": obsT_f,
        "embA": np.concatenate([emb.T, np.ones((1, L), FX)], 0),
        "wihA": np.concatenate([Wih.T, bih[None, :]], 0),
        "whhR": np.concatenate([Whh[0:H].T, bhh[None, 0:H]], 0),
        "whhZ": np.concatenate([Whh[H:2 * H].T, bhh[None, H:2 * H]], 0),
        "whhN": np.concatenate([Whh[2 * H:3 * H].T, bhh[None, 2 * H:3 * H]], 0),
        "f1w": np.asarray(f1_W, FX),
        "eyep": np.hstack([np.eye(H, dtype=FX), np.eye(H, dtype=FX)]),
    }

    def make_blob(percore):
        b = np.zeros((128, BLOBW), FX)
        for nm, (c0, w, r) in _BLOB.items():
            v = np.asarray(percore.get(nm, fields.get(nm)), FX)
            assert v.shape == (r, w), (nm, v.shape, (r, w))
            b[0:r, c0:c0 + w] = v
        return b

    common = {}
    obsHL = np.empty((128, 2 * N), BF)
    obsHL[:, 0::2] = obsH
    obsHL[:, 1::2] = obsL
    common["obsHL"] = np.ascontiguousarray(obsHL)
    common["obs8"] = np.ascontiguousarray(obsT_f.astype(F8))

    def pack(chunks, per_group, dtype):
        # [nchunk,128,RPC] -> [ngroup, 128, per_group*RPC], per-partition rows
        ng = chunks.shape[0] // per_group
        return np.ascontiguousarray(
            chunks.astype(dtype)
            .reshape(ng, per_group, 128, RPC).transpose(0, 2, 1, 3)
        ).reshape(ng, 128, per_group * RPC)

    in_maps = []
    for c in range(8):
        Wc = W1[RPC * c:RPC * (c + 1)].reshape(RPC, N, D + H + N)
        obs_part = Wc[:, :, 0:D].transpose(1, 2, 0)          # [256,128,64]
        g_part = Wc[:, :, D:D + H].transpose(1, 2, 0).reshape(128, 128, RPC)
        p0 = Wc[:, :, D + H:D + H + 128].transpose(1, 2, 0)  # [256,128,64]
        p1 = Wc[:, :, D + H + 128:].transpose(1, 2, 0)       # [256,128,64]
        p_part = np.concatenate([p0, p1], 0)                 # [512,128,64]
        w_hi = obs_part.astype(BF)                            # [256,128,64]
        w_lo = (obs_part - w_hi.astype(FX)) * LO_SCALE
        m = dict(common)
        m["wth"] = pack(w_hi, 128, BF)
        m["wtl"] = pack(w_lo, 256, F8)
        m["wtg"] = pack(g_part * G_WSCALE, 128, F8)
        m["wtp"] = pack(p_part * P_WSCALE, 256, F8)
        m["blob"] = make_blob({
            "b1row": b1[RPC * c:RPC * (c + 1)][None, :],
            "w2t": W2[:, RPC * c:RPC * (c + 1)].T,
            "b2col": (b2 if c == 0 else np.zeros_like(b2))[:, None],
        })
        in_maps.append(m)
    return in_maps


def kernel(**inputs) -> np.ndarray:
    if "nc" not in _CACHE:
        _CACHE["nc"] = _build_module()
    nc = _CACHE["nc"]
    in_maps = _prep_in_maps(**inputs)
    res = run_bass_kernel_spmd(nc, in_maps, list(range(8)))
    out = np.zeros((A, 1), FX)
    for r in res.results:
        out = out + r["out"]
    return out.reshape(A).astype(FX)


# revision 38
# speedup vs baseline: 1.0045x; 1.0045x over previous
"""Trainium2 Bass kernel for the DQN-style network.

Shapes (hardcoded): N=256, D=128, H=64, L=16, V=128, A=40,
F3_IN = N*(D+H+N) = 114688, f3 hidden = 512.

Strategy (8 cores, SPMD, no collectives):
  - Row-shard f3_W1: core c owns rows [64c, 64c+64).  Each core computes
    h1_c = relu(W1_c @ x + b1_c) fully locally, then its partial output
    W2[:, rows_c] @ h1_c (b2 added on core 0 only via its input data).
    The host sums the 8 partial [40] vectors (unshard of a sum-sharded
    output).
  - The tiny encoder (GRU over 16 tokens) and the rank-1 factorized
    global softmax p = softmax(u) (x) softmax(v) are computed redundantly
    on every core and overlap with the weight streaming.
  - x is built on-chip as SBUF tiles whose columns are the 128-long
    K-chunks of x; the host pre-permutes W1_c^T rows into the same chunk
    order so the accumulating matmuls (lhsT = x column(s), rhs = W chunk
    [128,64]) stream the weight at full DMA rate.
  - Mixed precision keeps the streamed bytes at 11.6 MB/core (vs 29.4
    f32) with ~6e-5 output error: the error budget is dominated by the
    obs block (x values O(1)), so obs weights stream as bf16-hi plus a
    2^13-scaled fp8 residual (with x split hi/lo bf16 + fp8), while the
    g/p blocks (tiny contributions) stream as scaled fp8.  Partial sums
    accumulate in three f32 PSUM accumulators (scales 1, 2^13, 2^24)
    that are descaled and combined in the tail.
"""

from contextlib import ExitStack

import numpy as np

import concourse.bass as bass
import concourse.mybir as mybir
from concourse import bacc
from concourse.bass_utils import run_bass_kernel_spmd
from concourse.masks import make_identity
from concourse.tile import TileContext

N, D, H, L, V, A = 256, 128, 64, 16, 128, 40
HID = 512                      # f3 hidden
RPC = HID // 8                 # rows of W1 per core = 64
NCHUNK = 896                   # 114688 / 128
NGROUP = 28                    # chunk groups of 32 (1 MB DMA each)
F32 = mybir.dt.float32
BF16 = mybir.dt.bfloat16
FP8 = mybir.dt.float8e4
FX = np.float32
LO_SCALE = 2.0 ** 13           # obs-residual fp8 pre-scale
G_WSCALE, G_XSCALE = 256.0, 32.0     # g-part fp8 pre-scales
P_WSCALE, P_XSCALE = 256.0, 65536.0  # p-part fp8 pre-scales

# blob free-dim column offsets: name -> (col0, width, rows)
_BLOB_FIELDS = [
    ("obsT", N, 128), ("embA", L, H + 1), ("wihA", 3 * H, H + 1),
    ("whhR", H, H + 1), ("whhZ", H, H + 1), ("whhN", H, H + 1),
    ("f1w", 2 * D, H), ("eyep", 128, H), ("b1row", RPC, 1),
    ("w2t", A, RPC), ("b2col", 1, A),
]
_BLOB = {}
_c = 0
for _nm, _w, _r in _BLOB_FIELDS:
    _BLOB[_nm] = (_c, _w, _r)
    _c += _w
BLOBW = _c

_CACHE = {}


def _build_module():
    nc = bacc.Bacc()

    wth = nc.declare_dram_parameter("wth", [2, 128, 8192], BF16, isOutput=False)
    wtl = nc.declare_dram_parameter("wtl", [1, 128, 16384], FP8, isOutput=False)
    wtg = nc.declare_dram_parameter("wtg", [1, 128, 8192], FP8, isOutput=False)
    wtp = nc.declare_dram_parameter("wtp", [2, 128, 16384], FP8, isOutput=False)
    obsHL = nc.declare_dram_parameter("obsHL", [128, 2 * N], BF16, isOutput=False)
    obs8 = nc.declare_dram_parameter("obs8", [128, N], FP8, isOutput=False)
    # all small f32 inputs packed into one [128, BLOBW] tensor (col ranges
    # in _BLOB): obsT | embA | wihA | whhR | whhZ | whhN | f1w | eyep |
    # b1row | w2t | b2col
    blob = nc.declare_dram_parameter("blob", [128, BLOBW], F32, isOutput=False)
    out = nc.declare_dram_parameter("out", [A, 1], F32, isOutput=True)

    AF = mybir.ActivationFunctionType
    OP = mybir.AluOpType

    with TileContext(nc) as tc, ExitStack() as st:
        cst = st.enter_context(tc.tile_pool(name="cst", bufs=1))
        tmp = st.enter_context(tc.tile_pool(name="tmp", bufs=2))
        wsp = st.enter_context(tc.tile_pool(name="wsp", bufs=6))
        psa = st.enter_context(tc.tile_pool(name="psa", bufs=1, space="PSUM"))
        psp = st.enter_context(tc.tile_pool(name="psp", bufs=4, space="PSUM"))

        # ---- constants / small input loads ----
        ident = cst.tile([128, 128], F32)
        make_identity(nc, ident[:, :])
        ones = cst.tile([128, 128], F32)
        nc.gpsimd.memset(ones[:, :], 1.0)

        blob_sb = cst.tile([128, BLOBW], F32)
        nc.sync.dma_start(out=blob_sb[:, :], in_=blob[:, :])
        wth_t0 = wsp.tile([128, 8192], BF16, tag="ws")
        nc.sync.dma_start(out=wth_t0[:, :], in_=wth[0, :, :])
        xhl = cst.tile([128, 2 * N], BF16)
        nc.sync.dma_start(out=xhl[:, :], in_=obsHL[:, :])
        x8 = cst.tile([128, N], FP8)
        nc.sync.dma_start(out=x8[:, :], in_=obs8[:, :])

        def bl(nm):
            c0, w, r = _BLOB[nm]
            return blob_sb[0:r, c0:c0 + w]

        xobs = bl("obsT")
        emb_sb = bl("embA")
        wih_sb = bl("wihA")
        whh_sb = {"r": bl("whhR"), "z": bl("whhZ"), "n": bl("whhN")}
        f1w_sb = bl("f1w")
        eyep_sb = bl("eyep")
        b1_sb = bl("b1row")
        w2t_sb = bl("w2t")
        b2_sb = bl("b2col")

        # ---- encoder: GI = [emb;1]^T @ [Wih^T;bih]  -> [L, 3H] ----
        ps_gi = psp.tile([L, 3 * H], F32, tag="ps")
        nc.tensor.matmul(ps_gi[:, :], emb_sb[:, :], wih_sb[:, :], start=True, stop=True)
        gi_sb = cst.tile([L, 3 * H], F32)
        nc.vector.tensor_copy(gi_sb[:, :], ps_gi[:, :])
        git = {}
        for j, nm in enumerate(("r", "z", "n")):
            ps_t = psp.tile([H, L], F32, tag="ps")
            nc.tensor.transpose(
                ps_t[:, :], gi_sb[:, j * H:(j + 1) * H], ident[0:L, 0:L]
            )
            g_sb = cst.tile([H, L], F32, tag=f"git_{nm}")
            nc.vector.tensor_copy(g_sb[:, :], ps_t[:, :])
            git[nm] = g_sb

        # ---- GRU: h kept as column in h_aug[0:64], h_aug[64] = 1 ----
        h_aug = cst.tile([H + 1, 1], F32)
        nc.gpsimd.memset(h_aug[:, :], 0.0)
        nc.gpsimd.memset(h_aug[H:H + 1, :], 1.0)
        for t in range(L):
            ps_r = psp.tile([H, 1], F32, tag="ps")
            nc.tensor.matmul(ps_r[:, :], whh_sb["r"][:, :], h_aug[:, :],
                             start=True, stop=True)
            r_s = tmp.tile([H, 1], F32, tag="r_s")
            nc.scalar.activation(r_s[:, :], ps_r[:, :], AF.Sigmoid,
                                 bias=git["r"][:, t:t + 1])
            ps_z = psp.tile([H, 1], F32, tag="ps")
            nc.tensor.matmul(ps_z[:, :], whh_sb["z"][:, :], h_aug[:, :],
                             start=True, stop=True)
            z_s = tmp.tile([H, 1], F32, tag="z_s")
            nc.scalar.activation(z_s[:, :], ps_z[:, :], AF.Sigmoid,
                                 bias=git["z"][:, t:t + 1])
            ps_n = psp.tile([H, 1], F32, tag="ps")
            nc.tensor.matmul(ps_n[:, :], whh_sb["n"][:, :], h_aug[:, :],
                             start=True, stop=True)
            # n = tanh(r * gh_n' + gi_n) in one ACT op (scale is [P,1] AP)
            n_s = tmp.tile([H, 1], F32, tag="n_s")
            nc.scalar.activation(n_s[:, :], ps_n[:, :], AF.Tanh,
                                 bias=git["n"][:, t:t + 1], scale=r_s[:, 0:1])
            t3 = tmp.tile([H, 1], F32, tag="t3")
            nc.vector.tensor_sub(t3[:, :], h_aug[0:H, :], n_s[:, :])
            # h' = (h - n) * z + n
            nc.vector.scalar_tensor_tensor(
                h_aug[0:H, :], t3[:, :], z_s[:, :], n_s[:, :],
                op0=OP.mult, op1=OP.add,
            )

        # ---- u = obs @ (Wa^T g), v = obs @ (Wb^T g) ----
        ps_wa = psp.tile([D, 1], F32, tag="ps")
        nc.tensor.matmul(ps_wa[:, :], f1w_sb[:, 0:D], h_aug[0:H, :],
                         start=True, stop=True)
        wa_s = tmp.tile([D, 1], F32, tag="wa_s")
        nc.vector.tensor_copy(wa_s[:, :], ps_wa[:, :])
        ps_wb = psp.tile([D, 1], F32, tag="ps")
        nc.tensor.matmul(ps_wb[:, :], f1w_sb[:, D:2 * D], h_aug[0:H, :],
                         start=True, stop=True)
        wb_s = tmp.tile([D, 1], F32, tag="wb_s")
        nc.vector.tensor_copy(wb_s[:, :], ps_wb[:, :])

        ps_u = psp.tile([128, 2], F32, tag="ps")
        nc.tensor.matmul(ps_u[:, 0:1], xobs[:, 0:128], wa_s[:, :],
                         start=True, stop=True)
        nc.tensor.matmul(ps_u[:, 1:2], xobs[:, 128:256], wa_s[:, :],
                         start=True, stop=True)
        eu = tmp.tile([128, 2], F32, tag="eu")
        nc.scalar.activation(eu[:, :], ps_u[:, :], AF.Exp)
        ps_v = psp.tile([128, 2], F32, tag="ps")
        nc.tensor.matmul(ps_v[:, 0:1], xobs[:, 0:128], wb_s[:, :],
                         start=True, stop=True)
        nc.tensor.matmul(ps_v[:, 1:2], xobs[:, 128:256], wb_s[:, :],
                         start=True, stop=True)
        ev = tmp.tile([128, 2], F32, tag="ev")
        nc.scalar.activation(ev[:, :], ps_v[:, :], AF.Exp)

        # global softmax normalizer: 1 / (sum(eu) * sum(ev))
        ps_su = psp.tile([1, 2], F32, tag="ps")
        nc.tensor.matmul(ps_su[:, :], ones[:, 0:1], eu[:, :], start=True, stop=True)
        ps_sv = psp.tile([1, 2], F32, tag="ps")
        nc.tensor.matmul(ps_sv[:, :], ones[:, 0:1], ev[:, :], start=True, stop=True)
        su_sb = tmp.tile([1, 2], F32, tag="su_sb")
        nc.vector.tensor_copy(su_sb[:, :], ps_su[:, :])
        sv_sb = tmp.tile([1, 2], F32, tag="sv_sb")
        nc.vector.tensor_copy(sv_sb[:, :], ps_sv[:, :])
        su_t = tmp.tile([1, 1], F32, tag="su_t")
        nc.vector.tensor_add(su_t[:, :], su_sb[:, 0:1], su_sb[:, 1:2])
        sv_t = tmp.tile([1, 1], F32, tag="sv_t")
        nc.vector.tensor_add(sv_t[:, :], sv_sb[:, 0:1], sv_sb[:, 1:2])
        sp_t = tmp.tile([1, 1], F32, tag="sp_t")
        nc.vector.tensor_mul(sp_t[:, :], su_t[:, :], sv_t[:, :])
        sc_t = tmp.tile([1, 1], F32, tag="sc_t")
        nc.vector.reciprocal(sc_t[:, :], sp_t[:, :])

        # rows: eu_row [1,256] scaled by sc; ev halves as [1,128] rows
        eu_row = tmp.tile([1, N], F32, tag="eu_row")
        for j in range(2):
            ps_t = psp.tile([1, 128], F32, tag="ps")
            nc.tensor.transpose(ps_t[:, :], eu[:, j:j + 1], ident[:, :])
            nc.vector.tensor_copy(eu_row[:, j * 128:(j + 1) * 128], ps_t[:, :])
        # fold the fp8 x-side pre-scale into the softmax normalizer
        eu_n = tmp.tile([1, N], F32, tag="eu_n")
        nc.vector.tensor_scalar(eu_n[:, :], eu_row[:, :], sc_t[:, 0:1],
                                P_XSCALE, op0=OP.mult, op1=OP.mult)
        ev_r = []
        for j in range(2):
            ps_t = psp.tile([1, 128], F32, tag="ps")
            nc.tensor.transpose(ps_t[:, :], ev[:, j:j + 1], ident[:, :])
            e_sb = tmp.tile([1, 128], F32, tag=f"ev_r{j}")
            nc.vector.tensor_copy(e_sb[:, :], ps_t[:, :])
            ev_r.append(e_sb)

        # p halves: outer products ev_half (x) (eu*sc*P_XSCALE), cast fp8
        xp8 = []
        for j in range(2):
            ps_p = psp.tile([128, N], F32, tag="ps")
            nc.tensor.matmul(ps_p[:, :], ev_r[j][:, :], eu_n[:, :],
                             start=True, stop=True)
            x_sb = cst.tile([128, N], FP8, tag=f"xp8_{j}")
            nc.vector.tensor_copy(x_sb[:, :], ps_p[:, :])
            xp8.append(x_sb)

        # g-pair tile: [g; g] * G_XSCALE broadcast to 128 columns, fp8
        ps_gp = psp.tile([128, 1], F32, tag="ps")
        nc.tensor.matmul(ps_gp[:, :], eyep_sb[:, :], h_aug[0:H, :],
                         start=True, stop=True)
        gcol = tmp.tile([128, 1], F32, tag="gcol")
        nc.vector.tensor_copy(gcol[:, :], ps_gp[:, :])
        xg8 = cst.tile([128, 128], FP8)
        nc.vector.tensor_scalar(xg8[:, :], ones[:, :], gcol[:, 0:1],
                                G_XSCALE, op0=OP.mult, op1=OP.mult)

        # ---- the big streamed matvec: 896 chunks of 128 ----
        # obs: W_hi bf16 (x hi+lo bf16 cols) -> ps_acc (scale 1)
        #      W_lo fp8*2^13 (x fp8)        -> ps8a (scale 2^13)
        # g:   W fp8*256 (x fp8*32)         -> ps8a (scale 2^13)
        # p:   W fp8*256 (x fp8*65536)      -> ps8b (scale 2^24)
        ps_acc = psa.tile([2, RPC], F32)
        ps8a = psa.tile([1, RPC], F32, tag="ps8a")
        ps8b = psa.tile([1, RPC], F32, tag="ps8b")
        for j in range(128):
            nc.tensor.matmul(
                ps_acc[:, :], xhl[:, 2 * j:2 * j + 2],
                wth_t0[:, j * 64:(j + 1) * 64],
                start=(j == 0), stop=False,
            )
        wtl_t = wsp.tile([128, 16384], FP8, tag="ws")
        nc.sync.dma_start(out=wtl_t[:, :], in_=wtl[0, :, :])
        for c in range(256):
            nc.tensor.matmul(
                ps8a[:, :], x8[:, c:c + 1], wtl_t[:, c * 64:(c + 1) * 64],
                start=(c == 0), stop=False,
            )
        for g in range(2):
            wtp_t = wsp.tile([128, 16384], FP8, tag="ws")
            nc.sync.dma_start(out=wtp_t[:, :], in_=wtp[g, :, :])
            for j in range(256):
                c = 256 * g + j
                xcol = xp8[0] if c < 256 else xp8[1]
                cc = c if c < 256 else c - 256
                nc.tensor.matmul(
                    ps8b[:, :], xcol[:, cc:cc + 1], wtp_t[:, j * 64:(j + 1) * 64],
                    start=(c == 0), stop=(c == 511),
                )
        wth_t1 = wsp.tile([128, 8192], BF16, tag="ws")
        nc.sync.dma_start(out=wth_t1[:, :], in_=wth[1, :, :])
        for j in range(128):
            c = 128 + j
            nc.tensor.matmul(
                ps_acc[:, :], xhl[:, 2 * c:2 * c + 2],
                wth_t1[:, j * 64:(j + 1) * 64],
                start=False, stop=(c == 255),
            )
        wtg_t = wsp.tile([128, 6144], FP8, tag="wsg0")
        nc.sync.dma_start(out=wtg_t[:, :], in_=wtg[0, :, 0:6144])
        for j in range(96):
            nc.tensor.matmul(
                ps8a[:, :], xg8[:, j:j + 1], wtg_t[:, j * 64:(j + 1) * 64],
                start=False, stop=False,
            )
        wtg_t1 = wsp.tile([128, 2048], FP8, tag="wsg1")
        nc.sync.dma_start(out=wtg_t1[:, :], in_=wtg[0, :, 6144:8192])
        for j in range(32):
            nc.tensor.matmul(
                ps8a[:, :], xg8[:, 96 + j:97 + j], wtg_t1[:, j * 64:(j + 1) * 64],
                start=False, stop=(j == 31),
            )

        # ---- tail: h1_c = relu(acc + b1_c); out = W2_c @ h1_c + b2 ----
        accs = tmp.tile([2, RPC], F32, tag="accs")
        nc.vector.tensor_copy(accs[:, :], ps_acc[:, :])
        ps_s = psp.tile([1, RPC], F32, tag="ps")
        nc.tensor.matmul(ps_s[:, :], ones[0:2, 0:1], accs[:, :],
                         start=True, stop=True)
        s1 = tmp.tile([1, RPC], F32, tag="s1")
        nc.vector.tensor_add(s1[:, :], ps_s[:, :], b1_sb[:, :])
        s2 = tmp.tile([1, RPC], F32, tag="s2")
        nc.vector.scalar_tensor_tensor(s2[:, :], ps8a[:, :], 1.0 / LO_SCALE,
                                       s1[:, :], op0=OP.mult, op1=OP.add)
        s3 = tmp.tile([1, RPC], F32, tag="s3")
        nc.vector.scalar_tensor_tensor(
            s3[:, :], ps8b[:, :], 1.0 / (P_WSCALE * P_XSCALE),
            s2[:, :], op0=OP.mult, op1=OP.add)
        h1r = tmp.tile([1, RPC], F32, tag="h1r")
        nc.scalar.activation(h1r[:, :], s3[:, :], AF.Relu)
        ps_h1 = psp.tile([RPC, 1], F32, tag="ps")
        nc.tensor.transpose(ps_h1[:, :], h1r[:, :], ident[0:1, 0:1])
        h1c = tmp.tile([RPC, 1], F32, tag="h1c")
        nc.vector.tensor_copy(h1c[:, :], ps_h1[:, :])
        ps_o = psp.tile([A, 1], F32, tag="ps")
        nc.tensor.matmul(ps_o[:, :], w2t_sb[:, :], h1c[:, :], start=True, stop=True)
        o_sb = tmp.tile([A, 1], F32, tag="o_sb")
        nc.vector.tensor_add(o_sb[:, :], ps_o[:, :], b2_sb[:, :])
        nc.sync.dma_start(out=out[:, :], in_=o_sb[:, :])

    nc.compile()
    return nc


def _prep_in_maps(obs, token_ids, emb_table, gru_Wih, gru_Whh, gru_bih,
                  gru_bhh, f1_W, f1_b, f3_W1, f3_b1, f3_W2, f3_b2):
    obs = np.asarray(obs, FX)
    ids = np.asarray(token_ids).astype(np.int64)
    emb = np.asarray(emb_table, FX)[ids]                     # [L, H]
    Wih = np.asarray(gru_Wih, FX)
    Whh = np.asarray(gru_Whh, FX)
    bih = np.asarray(gru_bih, FX)
    bhh = np.asarray(gru_bhh, FX)
    W1 = np.asarray(f3_W1, FX)
    W2 = np.asarray(f3_W2, FX)
    b1 = np.asarray(f3_b1, FX)
    b2 = np.asarray(f3_b2, FX)

    import ml_dtypes
    BF = ml_dtypes.bfloat16
    F8 = ml_dtypes.float8_e4m3

    obsT_f = np.ascontiguousarray(obs.T)
    obsH = obsT_f.astype(BF)
    obsL = (obsT_f - obsH.astype(FX)).astype(BF)
    fields = {
        "obsT": obsT_f,
        "embA": np.concatenate([emb.T, np.ones((1, L), FX)], 0),
        "wihA": np.concatenate([Wih.T, bih[None, :]], 0),
        "whhR": np.concatenate([Whh[0:H].T, bhh[None, 0:H]], 0),
        "whhZ": np.concatenate([Whh[H:2 * H].T, bhh[None, H:2 * H]], 0),
        "whhN": np.concatenate([Whh[2 * H:3 * H].T, bhh[None, 2 * H:3 * H]], 0),
        "f1w": np.asarray(f1_W, FX),
        "eyep": np.hstack([np.eye(H, dtype=FX), np.eye(H, dtype=FX)]),
    }

    def make_blob(percore):
        b = np.zeros((128, BLOBW), FX)
        for nm, (c0, w, r) in _BLOB.items():
            v = np.asarray(percore.get(nm, fields.get(nm)), FX)
            assert v.shape == (r, w), (nm, v.shape, (r, w))
            b[0:r, c0:c0 + w] = v
        return b

    common = {}
    obsHL = np.empty((128, 2 * N), BF)
    obsHL[:, 0::2] = obsH
    obsHL[:, 1::2] = obsL
    common["obsHL"] = np.ascontiguousarray(obsHL)
    common["obs8"] = np.ascontiguousarray(obsT_f.astype(F8))

    def pack(chunks, per_group, dtype):
        # [nchunk,128,RPC] -> [ngroup, 128, per_group*RPC], per-partition rows
        ng = chunks.shape[0] // per_group
        return np.ascontiguousarray(
            chunks.astype(dtype)
            .reshape(ng, per_group, 128, RPC).transpose(0, 2, 1, 3)
        ).reshape(ng, 128, per_group * RPC)

    in_maps = []
    for c in range(8):
        Wc = W1[RPC * c:RPC * (c + 1)].reshape(RPC, N, D + H + N)
        obs_part = Wc[:, :, 0:D].transpose(1, 2, 0)          # [256,128,64]
        g_part = Wc[:, :, D:D + H].transpose(1, 2, 0).reshape(128, 128, RPC)
        p0 = Wc[:, :, D + H:D + H + 128].transpose(1, 2, 0)  # [256,128,64]
        p1 = Wc[:, :, D + H + 128:].transpose(1, 2, 0)       # [256,128,64]
        p_part = np.concatenate([p0, p1], 0)                 # [512,128,64]
        w_hi = obs_part.astype(BF)                            # [256,128,64]
        w_lo = (obs_part - w_hi.astype(FX)) * LO_SCALE
        m = dict(common)
        m["wth"] = pack(w_hi, 128, BF)
        m["wtl"] = pack(w_lo, 256, F8)
        m["wtg"] = pack(g_part * G_WSCALE, 128, F8)
        m["wtp"] = pack(p_part * P_WSCALE, 256, F8)
        m["blob"] = make_blob({
            "b1row": b1[RPC * c:RPC * (c + 1)][None, :],
            "w2t": W2[:, RPC * c:RPC * (c + 1)].T,
            "b2col": (b2 if c == 0 else np.zeros_like(b2))[:, None],
        })
        in_maps.append(m)
    return in_maps


def kernel(**inputs) -> np.ndarray:
    if "nc" not in _CACHE:
        _CACHE["nc"] = _build_module()
    nc = _CACHE["nc"]
    in_maps = _prep_in_maps(**inputs)
    res = run_bass_kernel_spmd(nc, in_maps, list(range(8)))
    out = np.zeros((A, 1), FX)
    for r in res.results:
        out = out + r["out"]
    return out.reshape(A).astype(FX)


# revision 41
# speedup vs baseline: 1.1923x; 1.1869x over previous
"""Trainium2 Bass kernel for the DQN-style network.

Shapes (hardcoded): N=256, D=128, H=64, L=16, V=128, A=40,
F3_IN = N*(D+H+N) = 114688, f3 hidden = 512.

Strategy (8 cores, SPMD, no collectives):
  - Row-shard f3_W1: core c owns rows [64c, 64c+64).  Each core computes
    h1_c = relu(W1_c @ x + b1_c) fully locally, then its partial output
    W2[:, rows_c] @ h1_c (b2 added on core 0 only via its input data).
    The host sums the 8 partial [40] vectors (unshard of a sum-sharded
    output).
  - The tiny encoder (GRU over 16 tokens) and the rank-1 factorized
    global softmax p = softmax(u) (x) softmax(v) are computed redundantly
    on every core and overlap with the weight streaming.
  - x is built on-chip as SBUF tiles whose columns are the 128-long
    K-chunks of x; the host pre-permutes W1_c^T rows into the same chunk
    order so the accumulating matmuls (lhsT = x column(s), rhs = W chunk
    [128,64]) stream the weight at full DMA rate.
  - Mixed precision keeps the streamed bytes at 11.6 MB/core (vs 29.4
    f32) with ~6e-5 output error: the error budget is dominated by the
    obs block (x values O(1)), so obs weights stream as bf16-hi plus a
    2^13-scaled fp8 residual (with x split hi/lo bf16 + fp8), while the
    g/p blocks (tiny contributions) stream as scaled fp8.  Partial sums
    accumulate in three f32 PSUM accumulators (scales 1, 2^13, 2^24)
    that are descaled and combined in the tail.
"""

from contextlib import ExitStack

import numpy as np

import concourse.bass as bass
import concourse.mybir as mybir
from concourse import bacc
from concourse.bass_utils import run_bass_kernel_spmd
from concourse.masks import make_identity
from concourse.tile import TileContext

N, D, H, L, V, A = 256, 128, 64, 16, 128, 40
HID = 512                      # f3 hidden
RPC = HID // 8                 # rows of W1 per core = 64
NCHUNK = 896                   # 114688 / 128
NGROUP = 28                    # chunk groups of 32 (1 MB DMA each)
F32 = mybir.dt.float32
BF16 = mybir.dt.bfloat16
FP8 = mybir.dt.float8e4
FX = np.float32
LO_SCALE = 2.0 ** 13           # obs-residual fp8 pre-scale
G_WSCALE, G_XSCALE = 256.0, 32.0     # g-part fp8 pre-scales
P_WSCALE, P_XSCALE = 256.0, 65536.0  # p-part fp8 pre-scales

# blob free-dim column offsets: name -> (col0, width, rows)
_BLOB_FIELDS = [
    ("obsT", N, 128), ("embA", L, H + 1), ("wihA", 3 * H, H + 1),
    ("whhR", H, H + 1), ("whhZ", H, H + 1), ("whhN", H, H + 1),
    ("f1w", 2 * D, H), ("eyep", 128, H), ("b1row", RPC, 1),
    ("w2t", A, RPC), ("b2col", 1, A),
]
_BLOB = {}
_c = 0
for _nm, _w, _r in _BLOB_FIELDS:
    _BLOB[_nm] = (_c, _w, _r)
    _c += _w
BLOBW = _c

_CACHE = {}


def _build_module():
    nc = bacc.Bacc()

    wth = nc.declare_dram_parameter("wth", [2, 128, 8192], BF16, isOutput=False)
    wtl = nc.declare_dram_parameter("wtl", [1, 128, 16384], FP8, isOutput=False)
    wtg = nc.declare_dram_parameter("wtg", [1, 128, 8192], FP8, isOutput=False)
    wtp = nc.declare_dram_parameter("wtp", [2, 128, 16384], FP8, isOutput=False)
    obsHL = nc.declare_dram_parameter("obsHL", [128, 2 * N], BF16, isOutput=False)
    obs8 = nc.declare_dram_parameter("obs8", [128, N], FP8, isOutput=False)
    # all small f32 inputs packed into one [128, BLOBW] tensor (col ranges
    # in _BLOB): obsT | embA | wihA | whhR | whhZ | whhN | f1w | eyep |
    # b1row | w2t | b2col
    blob = nc.declare_dram_parameter("blob", [128, BLOBW], F32, isOutput=False)
    out = nc.declare_dram_parameter("out", [A, 1], F32, isOutput=True)

    AF = mybir.ActivationFunctionType
    OP = mybir.AluOpType

    with TileContext(nc) as tc, ExitStack() as st:
        cst = st.enter_context(tc.tile_pool(name="cst", bufs=1))
        tmp = st.enter_context(tc.tile_pool(name="tmp", bufs=2))
        wsp = st.enter_context(tc.tile_pool(name="wsp", bufs=6))
        psa = st.enter_context(tc.tile_pool(name="psa", bufs=1, space="PSUM"))
        psp = st.enter_context(tc.tile_pool(name="psp", bufs=4, space="PSUM"))

        # ---- constants / small input loads ----
        ident = cst.tile([128, 128], F32)
        make_identity(nc, ident[:, :])
        ones = cst.tile([128, 128], F32)
        nc.gpsimd.memset(ones[:, :], 1.0)

        blob_sb = cst.tile([128, BLOBW], F32)
        nc.sync.dma_start(out=blob_sb[:, :], in_=blob[:, :])
        wth_t0 = wsp.tile([128, 8192], BF16, tag="ws")
        nc.sync.dma_start(out=wth_t0[:, :], in_=wth[0, :, :])
        xhl = cst.tile([128, 2 * N], BF16)
        nc.sync.dma_start(out=xhl[:, :], in_=obsHL[:, :])
        x8 = cst.tile([128, N], FP8)
        nc.sync.dma_start(out=x8[:, :], in_=obs8[:, :])
        x8p = cst.tile([128, N, 16], FP8)
        nc.vector.tensor_copy(x8p[:, :, 0:1],
                              x8[:, :].rearrange("p (c m) -> p c m", m=1))

        def bl(nm):
            c0, w, r = _BLOB[nm]
            return blob_sb[0:r, c0:c0 + w]

        xobs = bl("obsT")
        emb_sb = bl("embA")
        wih_sb = bl("wihA")
        whh_sb = {"r": bl("whhR"), "z": bl("whhZ"), "n": bl("whhN")}
        f1w_sb = bl("f1w")
        eyep_sb = bl("eyep")
        b1_sb = bl("b1row")
        w2t_sb = bl("w2t")
        b2_sb = bl("b2col")

        # ---- encoder: GI = [emb;1]^T @ [Wih^T;bih]  -> [L, 3H] ----
        ps_gi = psp.tile([L, 3 * H], F32, tag="ps")
        nc.tensor.matmul(ps_gi[:, :], emb_sb[:, :], wih_sb[:, :], start=True, stop=True)
        gi_sb = cst.tile([L, 3 * H], F32)
        nc.vector.tensor_copy(gi_sb[:, :], ps_gi[:, :])
        git = {}
        for j, nm in enumerate(("r", "z", "n")):
            ps_t = psp.tile([H, L], F32, tag="ps")
            nc.tensor.transpose(
                ps_t[:, :], gi_sb[:, j * H:(j + 1) * H], ident[0:L, 0:L]
            )
            g_sb = cst.tile([H, L], F32, tag=f"git_{nm}")
            nc.vector.tensor_copy(g_sb[:, :], ps_t[:, :])
            git[nm] = g_sb

        # ---- GRU: h kept as column in h_aug[0:64], h_aug[64] = 1 ----
        h_aug = cst.tile([H + 1, 1], F32)
        nc.gpsimd.memset(h_aug[:, :], 0.0)
        nc.gpsimd.memset(h_aug[H:H + 1, :], 1.0)
        for t in range(L):
            ps_r = psp.tile([H, 1], F32, tag="ps")
            nc.tensor.matmul(ps_r[:, :], whh_sb["r"][:, :], h_aug[:, :],
                             start=True, stop=True)
            r_s = tmp.tile([H, 1], F32, tag="r_s")
            nc.scalar.activation(r_s[:, :], ps_r[:, :], AF.Sigmoid,
                                 bias=git["r"][:, t:t + 1])
            ps_z = psp.tile([H, 1], F32, tag="ps")
            nc.tensor.matmul(ps_z[:, :], whh_sb["z"][:, :], h_aug[:, :],
                             start=True, stop=True)
            z_s = tmp.tile([H, 1], F32, tag="z_s")
            nc.scalar.activation(z_s[:, :], ps_z[:, :], AF.Sigmoid,
                                 bias=git["z"][:, t:t + 1])
            ps_n = psp.tile([H, 1], F32, tag="ps")
            nc.tensor.matmul(ps_n[:, :], whh_sb["n"][:, :], h_aug[:, :],
                             start=True, stop=True)
            # n = tanh(r * gh_n' + gi_n) in one ACT op (scale is [P,1] AP)
            n_s = tmp.tile([H, 1], F32, tag="n_s")
            nc.scalar.activation(n_s[:, :], ps_n[:, :], AF.Tanh,
                                 bias=git["n"][:, t:t + 1], scale=r_s[:, 0:1])
            t3 = tmp.tile([H, 1], F32, tag="t3")
            nc.vector.tensor_sub(t3[:, :], h_aug[0:H, :], n_s[:, :])
            # h' = (h - n) * z + n
            nc.vector.scalar_tensor_tensor(
                h_aug[0:H, :], t3[:, :], z_s[:, :], n_s[:, :],
                op0=OP.mult, op1=OP.add,
            )

        # ---- u = obs @ (Wa^T g), v = obs @ (Wb^T g) ----
        ps_wa = psp.tile([D, 1], F32, tag="ps")
        nc.tensor.matmul(ps_wa[:, :], f1w_sb[:, 0:D], h_aug[0:H, :],
                         start=True, stop=True)
        wa_s = tmp.tile([D, 1], F32, tag="wa_s")
        nc.vector.tensor_copy(wa_s[:, :], ps_wa[:, :])
        ps_wb = psp.tile([D, 1], F32, tag="ps")
        nc.tensor.matmul(ps_wb[:, :], f1w_sb[:, D:2 * D], h_aug[0:H, :],
                         start=True, stop=True)
        wb_s = tmp.tile([D, 1], F32, tag="wb_s")
        nc.vector.tensor_copy(wb_s[:, :], ps_wb[:, :])

        ps_u = psp.tile([128, 2], F32, tag="ps")
        nc.tensor.matmul(ps_u[:, 0:1], xobs[:, 0:128], wa_s[:, :],
                         start=True, stop=True)
        nc.tensor.matmul(ps_u[:, 1:2], xobs[:, 128:256], wa_s[:, :],
                         start=True, stop=True)
        eu = tmp.tile([128, 2], F32, tag="eu")
        nc.scalar.activation(eu[:, :], ps_u[:, :], AF.Exp)
        ps_v = psp.tile([128, 2], F32, tag="ps")
        nc.tensor.matmul(ps_v[:, 0:1], xobs[:, 0:128], wb_s[:, :],
                         start=True, stop=True)
        nc.tensor.matmul(ps_v[:, 1:2], xobs[:, 128:256], wb_s[:, :],
                         start=True, stop=True)
        ev = tmp.tile([128, 2], F32, tag="ev")
        nc.scalar.activation(ev[:, :], ps_v[:, :], AF.Exp)

        # global softmax normalizer: 1 / (sum(eu) * sum(ev))
        ps_su = psp.tile([1, 2], F32, tag="ps")
        nc.tensor.matmul(ps_su[:, :], ones[:, 0:1], eu[:, :], start=True, stop=True)
        ps_sv = psp.tile([1, 2], F32, tag="ps")
        nc.tensor.matmul(ps_sv[:, :], ones[:, 0:1], ev[:, :], start=True, stop=True)
        su_sb = tmp.tile([1, 2], F32, tag="su_sb")
        nc.vector.tensor_copy(su_sb[:, :], ps_su[:, :])
        sv_sb = tmp.tile([1, 2], F32, tag="sv_sb")
        nc.vector.tensor_copy(sv_sb[:, :], ps_sv[:, :])
        su_t = tmp.tile([1, 1], F32, tag="su_t")
        nc.vector.tensor_add(su_t[:, :], su_sb[:, 0:1], su_sb[:, 1:2])
        sv_t = tmp.tile([1, 1], F32, tag="sv_t")
        nc.vector.tensor_add(sv_t[:, :], sv_sb[:, 0:1], sv_sb[:, 1:2])
        sp_t = tmp.tile([1, 1], F32, tag="sp_t")
        nc.vector.tensor_mul(sp_t[:, :], su_t[:, :], sv_t[:, :])
        sc_t = tmp.tile([1, 1], F32, tag="sc_t")
        nc.vector.reciprocal(sc_t[:, :], sp_t[:, :])

        # rows: eu_row [1,256] scaled by sc; ev halves as [1,128] rows
        eu_row = tmp.tile([1, N], F32, tag="eu_row")
        for j in range(2):
            ps_t = psp.tile([1, 128], F32, tag="ps")
            nc.tensor.transpose(ps_t[:, :], eu[:, j:j + 1], ident[:, :])
            nc.vector.tensor_copy(eu_row[:, j * 128:(j + 1) * 128], ps_t[:, :])
        # fold the fp8 x-side pre-scale into the softmax normalizer
        eu_n = tmp.tile([1, N], F32, tag="eu_n")
        nc.vector.tensor_scalar(eu_n[:, :], eu_row[:, :], sc_t[:, 0:1],
                                P_XSCALE, op0=OP.mult, op1=OP.mult)
        ev_r = []
        for j in range(2):
            ps_t = psp.tile([1, 128], F32, tag="ps")
            nc.tensor.transpose(ps_t[:, :], ev[:, j:j + 1], ident[:, :])
            e_sb = tmp.tile([1, 128], F32, tag=f"ev_r{j}")
            nc.vector.tensor_copy(e_sb[:, :], ps_t[:, :])
            ev_r.append(e_sb)

        # p halves: outer products ev_half (x) (eu*sc*P_XSCALE), cast fp8
        xp8 = []
        for j in range(2):
            ps_p = psp.tile([128, N], F32, tag="ps")
            nc.tensor.matmul(ps_p[:, :], ev_r[j][:, :], eu_n[:, :],
                             start=True, stop=True)
            x_sb = cst.tile([128, N, 16], FP8, tag=f"xp8_{j}")
            nc.vector.tensor_copy(x_sb[:, :, 0:1],
                                  ps_p[:, :].rearrange("p (c m) -> p c m", m=1))
            xp8.append(x_sb)

        # g-pair tile: [g; g] * G_XSCALE broadcast to 128 columns, fp8
        ps_gp = psp.tile([128, 1], F32, tag="ps")
        nc.tensor.matmul(ps_gp[:, :], eyep_sb[:, :], h_aug[0:H, :],
                         start=True, stop=True)
        gcol = tmp.tile([128, 1], F32, tag="gcol")
        nc.vector.tensor_copy(gcol[:, :], ps_gp[:, :])
        xg8 = cst.tile([128, 128], FP8)
        nc.vector.tensor_scalar(xg8[:, :], ones[:, :], gcol[:, 0:1],
                                G_XSCALE, op0=OP.mult, op1=OP.mult)
        xg8p = cst.tile([128, 128, 16], FP8)
        nc.vector.tensor_copy(xg8p[:, :, 0:1],
                              xg8[:, :].rearrange("p (c m) -> p c m", m=1))

        # ---- the big streamed matvec: 896 chunks of 128 ----
        # obs: W_hi bf16 (x hi+lo bf16 cols) -> ps_acc (scale 1)
        #      W_lo fp8*2^13 (x fp8)        -> ps8a (scale 2^13)
        # g:   W fp8*256 (x fp8*32)         -> ps8a (scale 2^13)
        # p:   W fp8*256 (x fp8*65536)      -> ps8b (scale 2^24)
        ps_acc = psa.tile([2, RPC], F32)
        ps8a = psa.tile([1, RPC], F32, tag="ps8a")
        ps8b = psa.tile([1, RPC], F32, tag="ps8b")
        for j in range(128):
            nc.tensor.matmul(
                ps_acc[:, :], xhl[:, 2 * j:2 * j + 2],
                wth_t0[:, j * 64:(j + 1) * 64],
                start=(j == 0), stop=False,
            )
        wtl_t = wsp.tile([128, 256, 64], FP8, tag="ws")
        nc.sync.dma_start(out=wtl_t[:, :, :],
                          in_=wtl[0, :, :].rearrange("p (c n) -> p c n", n=64))
        for d in range(128):
            nc.tensor.matmul(
                ps8a[:, :],
                x8p[:, 2 * d:2 * d + 2, 0:1],
                wtl_t[:, 2 * d:2 * d + 2, :],
                start=(d == 0), stop=False,
                perf_mode=mybir.MatmulPerfMode.DoubleRow,
            )
        for g in range(2):
            wtp_t = wsp.tile([128, 256, 64], FP8, tag="ws")
            nc.sync.dma_start(out=wtp_t[:, :, :],
                              in_=wtp[g, :, :].rearrange("p (c n) -> p c n", n=64))
            for j in range(128):
                d = 128 * g + j
                xcol = xp8[0] if d < 128 else xp8[1]
                nc.tensor.matmul(
                    ps8b[:, :],
                    xcol[:, 2 * j:2 * j + 2, 0:1],
                    wtp_t[:, 2 * j:2 * j + 2, :],
                    start=(d == 0), stop=(d == 255),
                    perf_mode=mybir.MatmulPerfMode.DoubleRow,
                )
        wth_t1 = wsp.tile([128, 8192], BF16, tag="ws")
        nc.sync.dma_start(out=wth_t1[:, :], in_=wth[1, :, :])
        for j in range(128):
            c = 128 + j
            nc.tensor.matmul(
                ps_acc[:, :], xhl[:, 2 * c:2 * c + 2],
                wth_t1[:, j * 64:(j + 1) * 64],
                start=False, stop=(c == 255),
            )
        wtg_t = wsp.tile([128, 96, 64], FP8, tag="wsg0")
        nc.sync.dma_start(out=wtg_t[:, :, :],
                          in_=wtg[0, :, 0:6144].rearrange("p (c n) -> p c n", n=64))
        for d in range(48):
            nc.tensor.matmul(
                ps8a[:, :],
                xg8p[:, 2 * d:2 * d + 2, 0:1],
                wtg_t[:, 2 * d:2 * d + 2, :],
                start=False, stop=False,
                perf_mode=mybir.MatmulPerfMode.DoubleRow,
            )
        wtg_t1 = wsp.tile([128, 32, 64], FP8, tag="wsg1")
        nc.sync.dma_start(out=wtg_t1[:, :, :],
                          in_=wtg[0, :, 6144:8192].rearrange("p (c n) -> p c n", n=64))
        for d in range(16):
            nc.tensor.matmul(
                ps8a[:, :],
                xg8p[:, 96 + 2 * d:98 + 2 * d, 0:1],
                wtg_t1[:, 2 * d:2 * d + 2, :],
                start=False, stop=(d == 15),
                perf_mode=mybir.MatmulPerfMode.DoubleRow,
            )

        # ---- tail: h1_c = relu(acc + b1_c); out = W2_c @ h1_c + b2 ----
        accs = tmp.tile([2, RPC], F32, tag="accs")
        nc.vector.tensor_copy(accs[:, :], ps_acc[:, :])
        ps_s = psp.tile([1, RPC], F32, tag="ps")
        nc.tensor.matmul(ps_s[:, :], ones[0:2, 0:1], accs[:, :],
                         start=True, stop=True)
        s1 = tmp.tile([1, RPC], F32, tag="s1")
        nc.vector.tensor_add(s1[:, :], ps_s[:, :], b1_sb[:, :])
        s2 = tmp.tile([1, RPC], F32, tag="s2")
        nc.vector.scalar_tensor_tensor(s2[:, :], ps8a[:, :], 1.0 / LO_SCALE,
                                       s1[:, :], op0=OP.mult, op1=OP.add)
        s3 = tmp.tile([1, RPC], F32, tag="s3")
        nc.vector.scalar_tensor_tensor(
            s3[:, :], ps8b[:, :], 1.0 / (P_WSCALE * P_XSCALE),
            s2[:, :], op0=OP.mult, op1=OP.add)
        h1r = tmp.tile([1, RPC], F32, tag="h1r")
        nc.scalar.activation(h1r[:, :], s3[:, :], AF.Relu)
        ps_h1 = psp.tile([RPC, 1], F32, tag="ps")
        nc.tensor.transpose(ps_h1[:, :], h1r[:, :], ident[0:1, 0:1])
        h1c = tmp.tile([RPC, 1], F32, tag="h1c")
        nc.vector.tensor_copy(h1c[:, :], ps_h1[:, :])
        ps_o = psp.tile([A, 1], F32, tag="ps")
        nc.tensor.matmul(ps_o[:, :], w2t_sb[:, :], h1c[:, :], start=True, stop=True)
        o_sb = tmp.tile([A, 1], F32, tag="o_sb")
        nc.vector.tensor_add(o_sb[:, :], ps_o[:, :], b2_sb[:, :])
        nc.sync.dma_start(out=out[:, :], in_=o_sb[:, :])

    nc.compile()
    return nc


def _prep_in_maps(obs, token_ids, emb_table, gru_Wih, gru_Whh, gru_bih,
                  gru_bhh, f1_W, f1_b, f3_W1, f3_b1, f3_W2, f3_b2):
    obs = np.asarray(obs, FX)
    ids = np.asarray(token_ids).astype(np.int64)
    emb = np.asarray(emb_table, FX)[ids]                     # [L, H]
    Wih = np.asarray(gru_Wih, FX)
    Whh = np.asarray(gru_Whh, FX)
    bih = np.asarray(gru_bih, FX)
    bhh = np.asarray(gru_bhh, FX)
    W1 = np.asarray(f3_W1, FX)
    W2 = np.asarray(f3_W2, FX)
    b1 = np.asarray(f3_b1, FX)
    b2 = np.asarray(f3_b2, FX)

    import ml_dtypes
    BF = ml_dtypes.bfloat16
    F8 = ml_dtypes.float8_e4m3

    obsT_f = np.ascontiguousarray(obs.T)
    obsH = obsT_f.astype(BF)
    obsL = (obsT_f - obsH.astype(FX)).astype(BF)
    fields = {
        "obsT": obsT_f,
        "embA": np.concatenate([emb.T, np.ones((1, L), FX)], 0),
        "wihA": np.concatenate([Wih.T, bih[None, :]], 0),
        "whhR": np.concatenate([Whh[0:H].T, bhh[None, 0:H]], 0),
        "whhZ": np.concatenate([Whh[H:2 * H].T, bhh[None, H:2 * H]], 0),
        "whhN": np.concatenate([Whh[2 * H:3 * H].T, bhh[None, 2 * H:3 * H]], 0),
        "f1w": np.asarray(f1_W, FX),
        "eyep": np.hstack([np.eye(H, dtype=FX), np.eye(H, dtype=FX)]),
    }

    def make_blob(percore):
        b = np.zeros((128, BLOBW), FX)
        for nm, (c0, w, r) in _BLOB.items():
            v = np.asarray(percore.get(nm, fields.get(nm)), FX)
            assert v.shape == (r, w), (nm, v.shape, (r, w))
            b[0:r, c0:c0 + w] = v
        return b

    common = {}
    obsHL = np.empty((128, 2 * N), BF)
    obsHL[:, 0::2] = obsH
    obsHL[:, 1::2] = obsL
    common["obsHL"] = np.ascontiguousarray(obsHL)
    common["obs8"] = np.ascontiguousarray(obsT_f.astype(F8))

    def pack(chunks, per_group, dtype):
        # [nchunk,128,RPC] -> [ngroup, 128, per_group*RPC], per-partition rows
        ng = chunks.shape[0] // per_group
        return np.ascontiguousarray(
            chunks.astype(dtype)
            .reshape(ng, per_group, 128, RPC).transpose(0, 2, 1, 3)
        ).reshape(ng, 128, per_group * RPC)

    in_maps = []
    for c in range(8):
        Wc = W1[RPC * c:RPC * (c + 1)].reshape(RPC, N, D + H + N)
        obs_part = Wc[:, :, 0:D].transpose(1, 2, 0)          # [256,128,64]
        g_part = Wc[:, :, D:D + H].transpose(1, 2, 0).reshape(128, 128, RPC)
        p0 = Wc[:, :, D + H:D + H + 128].transpose(1, 2, 0)  # [256,128,64]
        p1 = Wc[:, :, D + H + 128:].transpose(1, 2, 0)       # [256,128,64]
        p_part = np.concatenate([p0, p1], 0)                 # [512,128,64]
        w_hi = obs_part.astype(BF)                            # [256,128,64]
        w_lo = (obs_part - w_hi.astype(FX)) * LO_SCALE
        m = dict(common)
        m["wth"] = pack(w_hi, 128, BF)
        m["wtl"] = pack(w_lo, 256, F8)
        m["wtg"] = pack(g_part * G_WSCALE, 128, F8)
        m["wtp"] = pack(p_part * P_WSCALE, 256, F8)
        m["blob"] = make_blob({
            "b1row": b1[RPC * c:RPC * (c + 1)][None, :],
            "w2t": W2[:, RPC * c:RPC * (c + 1)].T,
            "b2col": (b2 if c == 0 else np.zeros_like(b2))[:, None],
        })
        in_maps.append(m)
    return in_maps


def kernel(**inputs) -> np.ndarray:
    if "nc" not in _CACHE:
        _CACHE["nc"] = _build_module()
    nc = _CACHE["nc"]
    in_maps = _prep_in_maps(**inputs)
    res = run_bass_kernel_spmd(nc, in_maps, list(range(8)))
    out = np.zeros((A, 1), FX)
    for r in res.results:
        out = out + r["out"]
    return out.reshape(A).astype(FX)
